# revision 4
# baseline (speedup 1.0000x reference)
"""MeteoGraphSAGE on 8 TRN2 cores — node(dst)-sharded, time-batched gathers.

Core k owns dst nodes [k*6272, (k+1)*6272) for ALL 8 timesteps.  Each
dma_gather descriptor fetches a node's features for all 8 timesteps at once
(1KB for x, 4KB for h1), cutting descriptor count 8x vs per-timestep
sharding.  BN stats all-reduce across cores; h1 gather table assembled via
AllGather; LSTM is node-parallel with no resharding.
"""

import math
import os
import sys

import numpy as np

for _p in ("/opt/trn_rl_repo", os.path.expanduser("~/.axon_site/_ro/trn_rl_repo")):
    if os.path.isdir(_p) and _p not in sys.path:
        sys.path.insert(0, _p)

import concourse.bacc as bacc
import concourse.bass as bass
import concourse.tile as tile
from concourse import bass_utils, mybir
from concourse.masks import make_identity

F32 = mybir.dt.float32
F16 = mybir.dt.float16
I16 = mybir.dt.int16
AF = mybir.ActivationFunctionType
OP = mybir.AluOpType

BN_EPS = 1e-5


class CFG:
    N, E = 50000, 1_600_000
    T, F, H, O = 8, 64, 256, 8
    NC = 8                       # cores
    BLK, BPC = 64, 7
    CHUNK = BLK * BPC            # 448
    CPS = 14                     # chunks per core
    NSHARD = CHUNK * CPS         # 6272 nodes per core
    NPAD = NSHARD * NC           # 50176
    NBLKC = NSHARD // BLK        # 98 blocks per core
    SPLIT = 32768
    TH = T * H                   # 2048
    TF = T * F                   # 512
    GA = 4                       # tiles per layer-1 gather call


FULL = CFG()


# ---------------------------------------------------------------- host prep
def host_prep(cfg, edge_index):
    c = cfg
    src = np.asarray(edge_index[0]).astype(np.int64)
    dst = np.asarray(edge_index[1]).astype(np.int64)
    counts = np.bincount(dst, minlength=c.N).astype(np.float32)
    recip = (1.0 / np.maximum(counts, 1.0)).astype(np.float32)
    z = (counts > 0).astype(np.float32)
    zpad = np.zeros(c.NPAD, np.float32)
    zpad[:c.N] = z

    per_core = []
    TA = TB = 1
    for k in range(c.NC):
        lo, hi = k * c.NSHARD, (k + 1) * c.NSHARD
        m = (dst >= lo) & (dst < hi)
        s_k, d_k = src[m], dst[m] - lo
        blk = d_k // c.BLK
        stream = (s_k >= c.SPLIT).astype(np.int64)
        key = blk * 2 + stream
        order = np.argsort(key, kind="stable")
        s_k, d_k, blk_k, st_k, key_k = (s_k[order], d_k[order], blk[order],
                                        stream[order], key[order])
        starts = np.searchsorted(key_k, np.arange(c.NBLKC * 2))
        ends = np.searchsorted(key_k, np.arange(c.NBLKC * 2) + 1)
        cnt = (ends - starts).reshape(c.NBLKC, 2)
        TA = max(TA, int(math.ceil(cnt[:, 0].max() / 128)))
        TB = max(TB, int(math.ceil(cnt[:, 1].max() / 128)))
        r = np.arange(len(s_k)) - starts[key_k]       # rank within (blk,stream)
        per_core.append(dict(s=s_k, d=d_k, blk=blk_k, st=st_k, r=r,
                             rc=recip[s_k * 0 + (d_k + lo)]))
    prep = dict(TA=TA, TB=TB, zpad=zpad, cores=[])
    for k in range(c.NC):
        pc = per_core[k]
        arrs = {}
        for sid, TS in ((0, TA), (1, TB)):
            m = pc["st"] == sid
            s_e, d_e, b_e, r_e = pc["s"][m], pc["d"][m], pc["blk"][m], pc["r"][m]
            rc_e = pc["rc"][m]
            idx = np.zeros((c.CPS, 128, c.BPC * TS * 8), np.int16)
            dl = np.full((c.CPS, 128, c.BPC * TS), -1.0, np.float32)
            rc = np.zeros((c.CPS, 128, c.BPC * TS), np.float32)
            ch = b_e // c.BPC
            bi = b_e % c.BPC
            # idx wrap: within (block,stream) sequence position r ->
            # row r%16 (replicated over 16-row bands), col bi*TS*8 + r//16
            rows16 = (r_e % 16).astype(np.int64)
            cols = bi * TS * 8 + r_e // 16
            val = np.where(sid == 0, s_e, s_e - c.SPLIT).astype(np.int16)
            idx[ch, rows16, cols] = val
            idx[:] = np.tile(idx[:, :16, :], (1, 8, 1))
            rowsd = (r_e % 128).astype(np.int64)
            cold = bi * TS + r_e // 128
            dl[ch, rowsd, cold] = (d_e % c.BLK).astype(np.float32)
            rc[ch, rowsd, cold] = rc_e
            sfx = "A" if sid == 0 else "B"
            arrs["idx" + sfx] = idx.reshape(c.CPS * 128, c.BPC * TS * 8)
            arrs["dl" + sfx] = dl.reshape(c.CPS * 128, c.BPC * TS)
            arrs["rc" + sfx] = rc.reshape(c.CPS * 128, c.BPC * TS)
        arrs["zv"] = zpad[None, k * c.NSHARD:(k + 1) * c.NSHARD].copy()
        arrs["npadn"] = np.full((128, 1), float(c.NPAD - c.N) if k == c.NC - 1
                                else 0.0, np.float32)
        prep["cores"].append(arrs)
    return prep


# ---------------------------------------------------------------- builder
def build_program(cfg, TA, TB, dbg=False):
    c = cfg
    T, F, H, O = c.T, c.F, c.H, c.O
    BLK, BPC, CHUNK, CPS = c.BLK, c.BPC, c.CHUNK, c.CPS
    NSHARD, TH, TF, GA = c.NSHARD, c.TH, c.TF, c.GA
    HT = H // 128                # 2
    GB = 4 * H // 128            # 8
    NGRP = [(TA, "A"), (TB, "B")]

    nc = bacc.Bacc("TRN2", target_bir_lowering=False, debug=False,
                   enable_asserts=False, num_devices=c.NC)
    dt = lambda n, s, d: nc.dram_tensor(n, s, d, kind="ExternalInput").ap()

    i_xTs = dt("xTs", [T, F, NSHARD], F32)
    i_xall = dt("xall", [c.NPAD, TF], F16)
    i_idxA = dt("idxA", [CPS * 128, BPC * TA * 8], I16)
    i_idxB = dt("idxB", [CPS * 128, BPC * TB * 8], I16)
    i_dlA = dt("dlA", [CPS * 128, BPC * TA], F32)
    i_dlB = dt("dlB", [CPS * 128, BPC * TB], F32)
    i_rcA = dt("rcA", [CPS * 128, BPC * TA], F32)
    i_rcB = dt("rcB", [CPS * 128, BPC * TB], F32)
    i_zv = dt("zv", [1, NSHARD], F32)
    i_iota = dt("iota", [128, BLK], F16)
    i_w0 = dt("w0", [F, H], F32)
    i_b0 = dt("b0v", [H], F32)
    i_wn0xE = dt("wn0xE", [128, H], F16)
    i_wn0xO = dt("wn0xO", [128, H], F16)
    i_b0wn = dt("b0wn", [1, H], F32)
    i_ws0 = dt("ws0", [H, H], F16)
    i_ws1 = dt("ws1", [H, H], F16)
    i_wn1 = dt("wn1", [H, H], F16)
    i_cb = [dt("cb0", [H], F32), dt("cb1", [H], F32)]
    i_g = [dt("g0", [H], F32), dt("g1", [H], F32)]
    i_bt = [dt("bt0", [H], F32), dt("bt1", [H], F32)]
    i_wih = dt("wih", [H, 4 * H], F32)
    i_whh = dt("whh", [H, 4 * H], F32)
    i_bg = dt("bg", [4 * H], F32)
    i_wdec = dt("wdec", [H, O], F32)
    i_bdec = dt("bdec", [O], F32)
    i_npadn = dt("npadn", [128, 1], F32)
    o_out = nc.dram_tensor("out", [O, NSHARD], F32, kind="ExternalOutput").ap()
    if dbg:
        o_dbga = [nc.dram_tensor("dbg_agg0", [128, 4 * CHUNK], F16,
                                 kind="ExternalOutput").ap(),
                  nc.dram_tensor("dbg_agg1", [128, 16 * CHUNK], F16,
                                 kind="ExternalOutput").ap()]
        o_dbgst = nc.dram_tensor("dbg_st", [32, 128], F32,
                                 kind="ExternalOutput").ap()
        o_dbgh = nc.dram_tensor("dbg_h1r", [128, TH], F16,
                                 kind="ExternalOutput").ap()
    grp = [list(range(c.NC))]

    with tile.TileContext(nc) as tc:
        with tc.tile_pool(name="dram", bufs=1, space="DRAM") as dp:
            h0 = dp.tile([T, H, NSHARD], F16)
            h1 = dp.tile([T, H, NSHARD], F16)
            h2 = dp.tile([T, H, NSHARD], F16)
            cmb = dp.tile([T, H, NSHARD], F16)
            h1loc = dp.tile([NSHARD, TH], F16)
            h1all = dp.tile([c.NPAD, TH], F16, addr_space="Shared")
            statsI = dp.tile([32, 128], F32)
            statsO0 = dp.tile([32, 128], F32, addr_space="Shared")
            statsO1 = dp.tile([32, 128], F32, addr_space="Shared")
            statsOL = [statsO0, statsO1]

        from contextlib import ExitStack
        _st = ExitStack()
        cp = _st.enter_context(tc.tile_pool(name="consts", bufs=1))
        ident = cp.tile([128, 128], F32)
        make_identity(nc, ident[:])
        identh = cp.tile([128, 128], F16)
        nc.vector.tensor_copy(out=identh[:], in_=ident[:])
        iota = cp.tile([128, BLK], F16)
        nc.sync.dma_start(out=iota[:], in_=i_iota[:, :])
        w0c = cp.tile([F, H], F32)
        nc.sync.dma_start(out=w0c[:], in_=i_w0[:, :])
        wn0x = [cp.tile([128, H], F16, name=f"wn0x{p}") for p in range(2)]
        nc.sync.dma_start(out=wn0x[0][:], in_=i_wn0xE[:, :])
        nc.sync.dma_start(out=wn0x[1][:], in_=i_wn0xO[:, :])
        b0wn = cp.tile([1, H], F32)
        nc.sync.dma_start(out=b0wn[:], in_=i_b0wn[:, :])
        wsk = {}
        for nm, t_in in (("ws0", i_ws0), ("ws1", i_ws1), ("wn1", i_wn1)):
            for k in range(HT):
                w = cp.tile([128, H], F16, name=f"{nm}k{k}")
                nc.sync.dma_start(out=w[:], in_=t_in[k * 128:(k + 1) * 128, :])
                wsk[nm, k] = w
        b0t, b0h, cbt, gt, btt = {}, {}, {}, {}, {}
        for m in range(HT):
            sl = slice(m * 128, (m + 1) * 128)
            b0t[m] = cp.tile([128, 1], F32, name=f"b0m{m}")
            nc.sync.dma_start(out=b0t[m][:], in_=i_b0[sl, None])
            b0h[m] = cp.tile([128, 1], F16, name=f"b0h{m}")
            nc.vector.tensor_copy(out=b0h[m][:], in_=b0t[m][:])
            for li in range(2):
                cbt[li, m] = cp.tile([128, 1], F32, name=f"cb{li}m{m}")
                nc.sync.dma_start(out=cbt[li, m][:], in_=i_cb[li][sl, None])
                gt[li, m] = cp.tile([128, 1], F32, name=f"g{li}m{m}")
                nc.sync.dma_start(out=gt[li, m][:], in_=i_g[li][sl, None])
                btt[li, m] = cp.tile([128, 1], F32, name=f"bt{li}m{m}")
                nc.sync.dma_start(out=btt[li, m][:], in_=i_bt[li][sl, None])
        npadnT = cp.tile([128, 1], F32)
        nc.sync.dma_start(out=npadnT[:], in_=i_npadn[:, :])

        sacc = _st.enter_context(tc.tile_pool(name="sacc", bufs=1))
        ssum = {(t, m): sacc.tile([128, 1], F32, name=f"ssum{t}_{m}")
                for t in range(T) for m in range(HT)}
        ssq = {(t, m): sacc.tile([128, 1], F32, name=f"ssq{t}_{m}")
               for t in range(T) for m in range(HT)}
        bnsc = {(t, m): sacc.tile([128, 1], F32, name=f"bnsc{t}_{m}")
                for t in range(T) for m in range(HT)}
        bnbi = {(t, m): sacc.tile([128, 1], F32, name=f"bnbi{t}_{m}")
                for t in range(T) for m in range(HT)}
        padh1 = {(t, m): sacc.tile([128, 1], F16, name=f"padh{t}_{m}")
                 for t in range(T) for m in range(HT)}
        epsT = sacc.tile([128, 1], F32, name="epsT")
        nc.vector.memset(epsT[:], BN_EPS)

        def init_stats():
            for t in range(T):
                for m in range(HT):
                    nc.vector.memset(ssum[t, m][:], 0.0)
                    nc.vector.memset(ssq[t, m][:], 0.0)

        # ================= P0: h0 = x @ W0 + b0 (per t, chunk-major f16)
        with tc.tile_pool(name="p0", bufs=3) as p0, \
                tc.tile_pool(name="p0ps", bufs=4, space="PSUM") as p0ps:
            for t in range(T):
                for ci in range(CPS):
                    csl = slice(ci * CHUNK, (ci + 1) * CHUNK)
                    xc = p0.tile([F, CHUNK], F32, tag="xc")
                    nc.sync.dma_start(out=xc[:], in_=i_xTs[t, :, csl])
                    for m in range(HT):
                        msl = slice(m * 128, (m + 1) * 128)
                        ps = p0ps.tile([128, CHUNK], F32, tag="ps")
                        nc.tensor.matmul(ps[:], lhsT=w0c[:, msl], rhs=xc[:],
                                         start=True, stop=True)
                        ho = p0.tile([128, CHUNK], F16, tag="ho")
                        nc.vector.tensor_scalar(ho[:], ps[:], b0t[m][:], 0.0,
                                                OP.add, OP.add)
                        nc.sync.dma_start(out=h0[t, msl, csl], in_=ho[:])

        # ================= pass1: gather + aggregate + comb + stats
        def pass1(li):
            ELEM = TF if li == 0 else TH            # f16 elems per desc
            table = i_xall if li == 0 else h1all
            hprev = h0 if li == 0 else h1
            idxs = {"A": i_idxA, "B": i_idxB}
            dls = {"A": i_dlA, "B": i_dlB}
            rcs = {"A": i_rcA, "B": i_rcB}
            ga = (max(TA, TB) if li == 0 else GA)   # tiles per gather call
            nacc = 4 if li == 0 else 16

            with tc.tile_pool(name=f"g{li}", bufs=3) as gp, \
                    tc.tile_pool(name=f"w{li}", bufs=2) as wp, \
                    tc.tile_pool(name=f"s{li}", bufs=6) as sp, \
                    tc.tile_pool(name=f"f{li}", bufs=2) as fp, \
                    tc.tile_pool(name=f"ps{li}", bufs=2, space="PSUM") as pp, \
                    tc.tile_pool(name=f"cps{li}", bufs=2, space="PSUM") as cpp:
                for ci in range(CPS):
                    csl = slice(ci * CHUNK, (ci + 1) * CHUNK)
                    r128 = ci * 128
                    st = {}
                    for TS, sfx in NGRP:
                        ix = fp.tile([128, BPC * TS * 8], I16, tag=f"ix{sfx}")
                        nc.sync.dma_start(out=ix[:],
                                          in_=idxs[sfx][r128:r128 + 128, :])
                        dl = fp.tile([128, BPC * TS], F32, tag=f"dl{sfx}")
                        nc.sync.dma_start(out=dl[:],
                                          in_=dls[sfx][r128:r128 + 128, :])
                        rc = fp.tile([128, BPC * TS], F32, tag=f"rc{sfx}")
                        nc.sync.dma_start(out=rc[:],
                                          in_=rcs[sfx][r128:r128 + 128, :])
                        st[sfx] = (ix, dl, rc)
                    hp = {}
                    for t in range(T):
                        for m in range(HT):
                            h_ = fp.tile([128, CHUNK], F16, tag=f"hp{t}{m}")
                            nc.sync.dma_start(
                                out=h_[:],
                                in_=hprev[t, m * 128:(m + 1) * 128, csl])
                            hp[t, m] = h_
                    agg = wp.tile([128, nacc, CHUNK], F16, tag="agg")
                    for b in range(BPC):
                        acc = pp.tile([128, nacc * BLK], F32, tag="accb",
                                      name="accb")
                        nc.vector.memset(acc[:], 0.0)
                        ntile = TA + TB
                        cur = 0
                        for TS, sfx in NGRP:
                            ix, dl, rc = st[sfx]
                            tb0 = 0 if sfx == "A" else c.SPLIT
                            ncall = (TS + ga - 1) // ga
                            for g in range(ncall):
                                nt = min(ga, TS - g * ga)
                                gtl = gp.tile([128, ga, ELEM], F16, tag="gt")
                                i0 = b * TS * 8 + g * ga * 8
                                nc.gpsimd.dma_gather(
                                    gtl[:, :nt, :], table[tb0:, :],
                                    ix[:, i0:i0 + nt * 8],
                                    nt * 128, nt * 128, ELEM,
                                    single_packet=False)
                                for j in range(nt):
                                    jj = b * TS + g * ga + j
                                    sel = sp.tile([128, BLK], F16, tag="sel")
                                    nc.vector.tensor_scalar(
                                        sel[:], iota[:], dl[:, jj:jj + 1],
                                        rc[:, jj:jj + 1], OP.is_equal, OP.mult)
                                    for i in range(nacc):
                                        nc.tensor.matmul(
                                            acc[:, i * BLK:(i + 1) * BLK],
                                            lhsT=gtl[:, j,
                                                     i * 128:(i + 1) * 128],
                                            rhs=sel[:],
                                            start=False,
                                            stop=(cur == ntile - 1))
                                    cur += 1
                        bs = slice(b * BLK, (b + 1) * BLK)
                        for i in range(nacc):
                            nc.vector.tensor_copy(
                                out=agg[:, i, bs],
                                in_=acc[:, i * BLK:(i + 1) * BLK])
                    if dbg and ci == 0:
                        nc.sync.dma_start(out=o_dbga[li][:, :],
                                          in_=agg[:, :, :])
                    # comb per (t, m)
                    for t in range(T):
                        for m in range(HT):
                            msl = slice(m * 128, (m + 1) * 128)
                            cps = cpp.tile([128, CHUNK], F32, tag="cps")
                            nm = "ws0" if li == 0 else "ws1"
                            nc.tensor.matmul(cps[:], lhsT=wsk[nm, 0][:, msl],
                                             rhs=hp[t, 0][:], start=True,
                                             stop=False)
                            nc.tensor.matmul(cps[:], lhsT=wsk[nm, 1][:, msl],
                                             rhs=hp[t, 1][:], start=False,
                                             stop=False)
                            if li == 0:
                                nc.tensor.matmul(cps[:],
                                                 lhsT=wn0x[t % 2][:, msl],
                                                 rhs=agg[:, t // 2, :],
                                                 start=False, stop=True)
                            else:
                                nc.tensor.matmul(cps[:],
                                                 lhsT=wsk["wn1", 0][:, msl],
                                                 rhs=agg[:, 2 * t, :],
                                                 start=False, stop=False)
                                nc.tensor.matmul(cps[:],
                                                 lhsT=wsk["wn1", 1][:, msl],
                                                 rhs=agg[:, 2 * t + 1, :],
                                                 start=False, stop=True)
                            c16 = wp.tile([128, CHUNK], F16, tag="c16")
                            tsum = sp.tile([128, 1], F32, tag="tsum")
                            nc.vector.tensor_scalar(c16[:], cps[:],
                                                    cbt[li, m][:], 0.0,
                                                    OP.add, OP.add,
                                                    accum_out=tsum[:])
                            nc.vector.tensor_add(ssum[t, m][:], ssum[t, m][:],
                                                 tsum[:])
                            sq = wp.tile([128, CHUNK], F32, tag="sq")
                            tsq = sp.tile([128, 1], F32, tag="tsq")
                            nc.scalar.activation(sq[:], cps[:], AF.Square,
                                                 bias=cbt[li, m][:], scale=1.0,
                                                 accum_out=tsq[:])
                            nc.vector.tensor_add(ssq[t, m][:], ssq[t, m][:],
                                                 tsq[:])
                            nc.sync.dma_start(out=cmb[t, msl, csl],
                                              in_=c16[:])

        # ================= BN finalize with cross-core stats AllReduce
        def bn_ar(li):
            with tc.tile_pool(name=f"bn{li}", bufs=2) as bp, \
                    tc.tile_pool(name=f"bnps{li}", bufs=2, space="PSUM") as bpp:
                nm = "ws0" if li == 0 else "ws1"
                padc = {}
                for t in range(T):
                    for m in range(HT):
                        msl = slice(m * 128, (m + 1) * 128)
                        if li == 0 and t > 0:
                            padc[t, m] = padc[0, m]
                            continue
                        pp_ = bpp.tile([128, 1], F32, tag="pp")
                        ph = ((b0h[0], b0h[1]) if li == 0
                              else (padh1[t, 0], padh1[t, 1]))
                        nc.tensor.matmul(pp_[:], lhsT=wsk[nm, 0][:, msl],
                                         rhs=ph[0][:], start=True, stop=False)
                        nc.tensor.matmul(pp_[:], lhsT=wsk[nm, 1][:, msl],
                                         rhs=ph[1][:], start=False, stop=True)
                        pc = bp.tile([128, 1], F32, name=f"padc{li}_{t}_{m}")
                        nc.vector.tensor_scalar(pc[:], pp_[:], cbt[li, m][:],
                                                0.0, OP.add, OP.add)
                        padc[t, m] = pc
                for t in range(T):
                    for m in range(HT):
                        r = t * HT + m
                        tmp = bp.tile([128, 1], F32, tag="tmp")
                        nc.vector.tensor_tensor(out=tmp[:], in0=padc[t, m][:],
                                                in1=npadnT[:], op=OP.mult)
                        nc.vector.tensor_tensor(out=ssum[t, m][:],
                                                in0=ssum[t, m][:], in1=tmp[:],
                                                op=OP.subtract)
                        sq2 = bp.tile([128, 1], F32, tag="sq2")
                        nc.scalar.activation(sq2[:], padc[t, m][:], AF.Square)
                        nc.vector.tensor_tensor(out=sq2[:], in0=sq2[:],
                                                in1=npadnT[:], op=OP.mult)
                        nc.vector.tensor_tensor(out=ssq[t, m][:],
                                                in0=ssq[t, m][:], in1=sq2[:],
                                                op=OP.subtract)
                        nc.sync.dma_start(out=statsI[r, :, None],
                                          in_=ssum[t, m][:])
                        nc.sync.dma_start(out=statsI[16 + r, :, None],
                                          in_=ssq[t, m][:])
                statsO = statsOL[li]
                nc.gpsimd.collective_compute(
                    "AllReduce", OP.add, replica_groups=grp,
                    ins=[statsI[:, :]], outs=[statsO[:, :]])
                if dbg and li == 0:
                    stile = bp.tile([32, 128], F32, name="stile")
                    nc.sync.dma_start(out=stile[:], in_=statsO[:, :])
                    nc.sync.dma_start(out=o_dbgst[:, :], in_=stile[:])
                for t in range(T):
                    for m in range(HT):
                        r = t * HT + m
                        rsum = bp.tile([128, 1], F32, tag="rsum")
                        nc.sync.dma_start(out=rsum[:], in_=statsO[r, :, None])
                        rsq = bp.tile([128, 1], F32, tag="rsq")
                        nc.sync.dma_start(out=rsq[:],
                                          in_=statsO[16 + r, :, None])
                        mu = bp.tile([128, 1], F32, tag="mu")
                        nc.vector.tensor_scalar(mu[:], rsum[:], 1.0 / c.N,
                                                0.0, OP.mult, OP.add)
                        var = bp.tile([128, 1], F32, tag="var")
                        nc.vector.tensor_scalar(var[:], rsq[:], 1.0 / c.N,
                                                0.0, OP.mult, OP.add)
                        musq = bp.tile([128, 1], F32, tag="musq")
                        nc.vector.tensor_tensor(out=musq[:], in0=mu[:],
                                                in1=mu[:], op=OP.mult)
                        nc.vector.tensor_tensor(out=var[:], in0=var[:],
                                                in1=musq[:], op=OP.subtract)
                        std = bp.tile([128, 1], F32, tag="std")
                        nc.scalar.activation(std[:], var[:], AF.Sqrt,
                                             bias=epsT[:])
                        rstd = bp.tile([128, 1], F32, tag="rstd")
                        nc.vector.reciprocal(rstd[:], std[:])
                        nc.vector.tensor_tensor(out=bnsc[t, m][:],
                                                in0=gt[li, m][:], in1=rstd[:],
                                                op=OP.mult)
                        mt = bp.tile([128, 1], F32, tag="mt")
                        nc.vector.tensor_tensor(out=mt[:], in0=mu[:],
                                                in1=bnsc[t, m][:], op=OP.mult)
                        nc.vector.tensor_tensor(out=bnbi[t, m][:],
                                                in0=btt[li, m][:], in1=mt[:],
                                                op=OP.subtract)
                        pr = bp.tile([128, 1], F32, tag="pr")
                        nc.scalar.activation(pr[:], padc[t, m][:], AF.Relu,
                                             bias=bnbi[t, m][:],
                                             scale=bnsc[t, m][:])
                        ph0 = b0h[m] if li == 0 else padh1[t, m]
                        nc.vector.tensor_add(padh1[t, m][:], ph0[:], pr[:])

        # ================= pass2: h_next = h_prev + relu(BN(cmb))
        def pass2(li):
            hprev = h0 if li == 0 else h1
            hnext = h1 if li == 0 else h2
            with tc.tile_pool(name=f"q{li}", bufs=3) as qp, \
                    tc.tile_pool(name=f"qn{li}", bufs=2) as qn, \
                    tc.tile_pool(name=f"qps{li}", bufs=4, space="PSUM") as qpp:
                for ci in range(CPS):
                    csl = slice(ci * CHUNK, (ci + 1) * CHUNK)
                    hn = {}
                    for t in range(T):
                        for m in range(HT):
                            msl = slice(m * 128, (m + 1) * 128)
                            c16 = qp.tile([128, CHUNK], F16, tag="c16")
                            nc.sync.dma_start(out=c16[:], in_=cmb[t, msl, csl])
                            h_ = qp.tile([128, CHUNK], F16, tag="hpv")
                            nc.sync.dma_start(out=h_[:],
                                              in_=hprev[t, msl, csl])
                            rl = qp.tile([128, CHUNK], F32, tag="rl")
                            nc.scalar.activation(rl[:], c16[:], AF.Relu,
                                                 bias=bnbi[t, m][:],
                                                 scale=bnsc[t, m][:])
                            if li == 0:
                                hx = qn.tile([128, CHUNK], F16,
                                             tag=f"hn{t}{m}")
                            else:
                                hx = qp.tile([128, CHUNK], F16, tag="hx")
                            nc.vector.tensor_add(hx[:], h_[:], rl[:])
                            hn[t, m] = hx
                            nc.sync.dma_start(out=hnext[t, msl, csl],
                                              in_=hx[:])
                    if li == 0:
                        # node-major h1loc rows for the AllGather table
                        for nb in range((CHUNK + 127) // 128):
                            w = min(128, CHUNK - nb * 128)
                            nsl = slice(nb * 128, nb * 128 + w)
                            n16 = qp.tile([128, TH], F16, tag="n16")
                            for t in range(T):
                                for m in range(HT):
                                    tp = qpp.tile([128, 128], F16, tag="tp")
                                    nc.tensor.transpose(
                                        tp[:w, :], hn[t, m][:, nsl],
                                        identh[:])
                                    nc.vector.tensor_copy(
                                        out=n16[:w, t * H + m * 128:
                                                t * H + (m + 1) * 128],
                                        in_=tp[:w, :])
                            r0 = ci * CHUNK + nb * 128
                            nc.sync.dma_start(out=h1loc[r0:r0 + w, :],
                                              in_=n16[:w, :])

        init_stats()
        pass1(0)
        bn_ar(0)
        pass2(0)
        nc.gpsimd.collective_compute(
            "AllGather", OP.bypass, replica_groups=grp,
            ins=[h1loc[:, :]], outs=[h1all[:, :]])
        if dbg:
            with tc.tile_pool(name="dbgh", bufs=1) as dbp:
                ht_ = dbp.tile([128, TH], F16)
                nc.sync.dma_start(out=ht_[:], in_=h1loc[0:128, :])
                nc.sync.dma_start(out=o_dbgh[:, :], in_=ht_[:])
        init_stats()
        pass1(1)
        bn_ar(1)
        pass2(1)

        _st.close()

        # ================= LSTM over time + decoder (node-parallel)
        NHALF = NSHARD // 2
        CH = NHALF // CHUNK
        with tc.tile_pool(name="lw", bufs=1) as lw, \
                tc.tile_pool(name="lst", bufs=1) as ls, \
                tc.tile_pool(name="lwk", bufs=3) as lk, \
                tc.tile_pool(name="lps", bufs=4, space="PSUM") as lp:
            wih = [lw.tile([128, 4 * H], F16, name=f"wih{k}") for k in range(HT)]
            whh = [lw.tile([128, 4 * H], F16, name=f"whh{k}") for k in range(HT)]
            for k in range(HT):
                nc.gpsimd.dma_start(out=wih[k][:],
                                    in_=i_wih[k * 128:(k + 1) * 128, :])
                nc.gpsimd.dma_start(out=whh[k][:],
                                    in_=i_whh[k * 128:(k + 1) * 128, :])
            bgt = [lw.tile([128, 1], F32, name=f"bg{g}") for g in range(GB)]
            for g in range(GB):
                nc.sync.dma_start(out=bgt[g][:],
                                  in_=i_bg[g * 128:(g + 1) * 128, None])
            bdt = lw.tile([O, 1], F32)
            nc.sync.dma_start(out=bdt[:], in_=i_bdec[:, None])
            wdt = [lw.tile([128, O], F16, name=f"wd{k}") for k in range(HT)]
            for k in range(HT):
                nc.gpsimd.dma_start(out=wdt[k][:],
                                    in_=i_wdec[k * 128:(k + 1) * 128, :])

            cst = [ls.tile([128, NSHARD], F32, name=f"c{m}") for m in range(HT)]
            hst = [ls.tile([128, NSHARD], F16, name=f"h{m}") for m in range(HT)]
            gst = [ls.tile([128, NHALF], F16, name=f"gs{g}") for g in range(GB)]
            eh = [ls.tile([128, NHALF], F16, name=f"e{k}") for k in range(HT)]
            for m in range(HT):
                nc.vector.memset(cst[m][:], 0.0)
                nc.vector.memset(hst[m][:], 0.0)

            for step in range(T):
                for half in range(2):
                    hoff = half * NHALF
                    for k in range(HT):
                        nc.sync.dma_start(
                            out=eh[k][:],
                            in_=h2[step, k * 128:(k + 1) * 128,
                                   hoff:hoff + NHALF])
                    for g in range(GB):
                        gsl = slice(g * 128, (g + 1) * 128)
                        fn = AF.Tanh if g in (4, 5) else AF.Sigmoid
                        for ch in range(CH):
                            s0, s1 = ch * CHUNK, (ch + 1) * CHUNK
                            ps = lp.tile([128, CHUNK], F32, tag="gps")
                            nc.tensor.matmul(ps[:], lhsT=wih[0][:, gsl],
                                             rhs=eh[0][:, s0:s1], start=True,
                                             stop=False)
                            nc.tensor.matmul(ps[:], lhsT=wih[1][:, gsl],
                                             rhs=eh[1][:, s0:s1], start=False,
                                             stop=False)
                            nc.tensor.matmul(
                                ps[:], lhsT=whh[0][:, gsl],
                                rhs=hst[0][:, hoff + s0:hoff + s1],
                                start=False, stop=False)
                            nc.tensor.matmul(
                                ps[:], lhsT=whh[1][:, gsl],
                                rhs=hst[1][:, hoff + s0:hoff + s1],
                                start=False, stop=True)
                            nc.scalar.activation(gst[g][:, s0:s1], ps[:], fn,
                                                 bias=bgt[g][:])
                    for ch in range(CH):
                        s0, s1 = ch * CHUNK, (ch + 1) * CHUNK
                        for m in range(HT):
                            csl_ = cst[m][:, hoff + s0:hoff + s1]
                            t1 = lk.tile([128, CHUNK], F32, tag="t1")
                            nc.vector.tensor_tensor(
                                out=t1[:], in0=gst[2 + m][:, s0:s1],
                                in1=csl_, op=OP.mult)
                            t2 = lk.tile([128, CHUNK], F32, tag="t2")
                            nc.vector.tensor_tensor(
                                out=t2[:], in0=gst[0 + m][:, s0:s1],
                                in1=gst[4 + m][:, s0:s1], op=OP.mult)
                            nc.vector.tensor_tensor(out=csl_, in0=t1[:],
                                                    in1=t2[:], op=OP.add)
                            t3 = lk.tile([128, CHUNK], F32, tag="t3")
                            nc.scalar.activation(t3[:], csl_, AF.Tanh)
                            nc.vector.tensor_tensor(
                                out=hst[m][:, hoff + s0:hoff + s1],
                                in0=gst[6 + m][:, s0:s1], in1=t3[:],
                                op=OP.mult)
            for ci in range(CPS):
                s0, s1 = ci * CHUNK, (ci + 1) * CHUNK
                ps = lp.tile([O, CHUNK], F32, tag="dps")
                nc.tensor.matmul(ps[:], lhsT=wdt[0][:], rhs=hst[0][:, s0:s1],
                                 start=True, stop=False)
                nc.tensor.matmul(ps[:], lhsT=wdt[1][:], rhs=hst[1][:, s0:s1],
                                 start=False, stop=True)
                ob = lk.tile([O, CHUNK], F32, tag="ob")
                nc.vector.tensor_scalar(ob[:], ps[:], bdt[:], 0.0, OP.add,
                                        OP.add)
                nc.sync.dma_start(out=o_out[:, s0:s1], in_=ob[:])

    nc.compile()
    return nc


# ---------------------------------------------------------------- driver
def _make_in_maps(cfg, prep, x, W0, b0, Ws_self, bs_self, Ws_nei, bs_nei,
                  gamma, beta, W_ih, W_hh, b_ih, b_hh, W_dec, b_dec):
    c = cfg
    x = np.asarray(x, np.float32)
    W0 = np.asarray(W0, np.float32)
    b0 = np.asarray(b0, np.float32)
    Ws_nei = np.asarray(Ws_nei, np.float32)
    wn0x = (W0 @ Ws_nei[0]).astype(np.float16)          # [64, 256]
    zr64 = np.zeros((64, c.H), np.float16)
    common = dict(
        xall=None,
        iota=np.broadcast_to(np.arange(c.BLK, dtype=np.float16),
                             (128, c.BLK)).copy(),
        w0=W0, b0v=b0,
        wn0xE=np.concatenate([wn0x, zr64], axis=0),
        wn0xO=np.concatenate([zr64, wn0x], axis=0),
        b0wn=(b0 @ Ws_nei[0])[None, :].astype(np.float32),
        ws0=np.asarray(Ws_self[0], np.float16),
        ws1=np.asarray(Ws_self[1], np.float16),
        wn1=Ws_nei[1].astype(np.float16),
        cb0=(np.asarray(bs_self[0]) + np.asarray(bs_nei[0])).astype(np.float32),
        cb1=(np.asarray(bs_self[1]) + np.asarray(bs_nei[1])).astype(np.float32),
        g0=np.asarray(gamma[0], np.float32),
        g1=np.asarray(gamma[1], np.float32),
        bt0=np.asarray(beta[0], np.float32),
        bt1=np.asarray(beta[1], np.float32),
        wih=np.ascontiguousarray(np.asarray(W_ih, np.float32).T),
        whh=np.ascontiguousarray(np.asarray(W_hh, np.float32).T),
        bg=(np.asarray(b_ih) + np.asarray(b_hh)).astype(np.float32),
        wdec=np.asarray(W_dec, np.float32),
        bdec=np.asarray(b_dec, np.float32),
    )
    # x_all gather table: node-major, all timesteps concat [NPAD, T*F] f16
    xall = np.zeros((c.NPAD, c.TF), np.float16)
    xall[:c.N] = x.transpose(1, 0, 2).reshape(c.N, c.TF).astype(np.float16)
    common["xall"] = xall
    in_maps = []
    for k in range(c.NC):
        lo, hi = k * c.NSHARD, (k + 1) * c.NSHARD
        xs = np.zeros((c.T, c.F, c.NSHARD), np.float32)
        n_real = max(0, min(hi, c.N) - lo)
        if n_real > 0:
            xs[:, :, :n_real] = x[:, lo:lo + n_real, :].transpose(0, 2, 1)
        im = dict(common, xTs=xs, **prep["cores"][k])
        in_maps.append(im)
    return in_maps


def run(cfg, inputs, trace=False):
    prep = host_prep(cfg, inputs["edge_index"])
    nc = build_program(cfg, prep["TA"], prep["TB"])
    in_maps = _make_in_maps(cfg, prep, **{k: v for k, v in inputs.items()
                                          if k != "edge_index"})
    res = bass_utils.run_bass_kernel_spmd(
        nc, in_maps, core_ids=list(range(cfg.NC)), trace=trace)
    outs = [res.results[c]["out"] for c in range(cfg.NC)]
    full = np.concatenate(outs, axis=1)          # [O, NPAD]
    return np.ascontiguousarray(full.T[:cfg.N]), res


def kernel(**inputs):
    out, _ = run(FULL, inputs, trace=bool(os.environ.get("BASS_TRACE")))
    return out.astype(np.float32)


# revision 5
# speedup vs baseline: 1.0398x; 1.0398x over previous
"""MeteoGraphSAGE on 8 TRN2 cores — node(dst)-sharded, time-batched gathers.

Core k owns dst nodes [k*6272, (k+1)*6272) for ALL 8 timesteps.  Each
dma_gather descriptor fetches a node's features for all 8 timesteps at once
(1KB for x, 4KB for h1), cutting descriptor count 8x vs per-timestep
sharding.  BN stats all-reduce across cores; h1 gather table assembled via
AllGather; LSTM is node-parallel with no resharding.
"""

import math
import os
import sys

import numpy as np

for _p in ("/opt/trn_rl_repo", os.path.expanduser("~/.axon_site/_ro/trn_rl_repo")):
    if os.path.isdir(_p) and _p not in sys.path:
        sys.path.insert(0, _p)

import concourse.bacc as bacc
import concourse.bass as bass
import concourse.tile as tile
from concourse import bass_utils, mybir
from concourse.masks import make_identity

F32 = mybir.dt.float32
F16 = mybir.dt.float16
I16 = mybir.dt.int16
AF = mybir.ActivationFunctionType
OP = mybir.AluOpType

BN_EPS = 1e-5


class CFG:
    N, E = 50000, 1_600_000
    T, F, H, O = 8, 64, 256, 8
    NC = 8                       # cores
    BLK, BPC = 64, 7
    CHUNK = BLK * BPC            # 448
    CPS = 14                     # chunks per core
    NSHARD = CHUNK * CPS         # 6272 nodes per core
    NPAD = NSHARD * NC           # 50176
    NBLKC = NSHARD // BLK        # 98 blocks per core
    SPLIT = 32768
    TH = T * H                   # 2048
    TF = T * F                   # 512
    GA = 4                       # tiles per layer-1 gather call


FULL = CFG()


# ---------------------------------------------------------------- host prep
def host_prep(cfg, edge_index):
    c = cfg
    src = np.asarray(edge_index[0]).astype(np.int64)
    dst = np.asarray(edge_index[1]).astype(np.int64)
    counts = np.bincount(dst, minlength=c.N).astype(np.float32)
    recip = (1.0 / np.maximum(counts, 1.0)).astype(np.float32)
    z = (counts > 0).astype(np.float32)
    zpad = np.zeros(c.NPAD, np.float32)
    zpad[:c.N] = z

    per_core = []
    TA = TB = 1
    for k in range(c.NC):
        lo, hi = k * c.NSHARD, (k + 1) * c.NSHARD
        m = (dst >= lo) & (dst < hi)
        s_k, d_k = src[m], dst[m] - lo
        blk = d_k // c.BLK
        stream = (s_k >= c.SPLIT).astype(np.int64)
        key = blk * 2 + stream
        order = np.argsort(key, kind="stable")
        s_k, d_k, blk_k, st_k, key_k = (s_k[order], d_k[order], blk[order],
                                        stream[order], key[order])
        starts = np.searchsorted(key_k, np.arange(c.NBLKC * 2))
        ends = np.searchsorted(key_k, np.arange(c.NBLKC * 2) + 1)
        cnt = (ends - starts).reshape(c.NBLKC, 2)
        TA = max(TA, int(math.ceil(cnt[:, 0].max() / 128)))
        TB = max(TB, int(math.ceil(cnt[:, 1].max() / 128)))
        r = np.arange(len(s_k)) - starts[key_k]       # rank within (blk,stream)
        per_core.append(dict(s=s_k, d=d_k, blk=blk_k, st=st_k, r=r,
                             rc=recip[s_k * 0 + (d_k + lo)]))
    prep = dict(TA=TA, TB=TB, zpad=zpad, cores=[])
    for k in range(c.NC):
        pc = per_core[k]
        arrs = {}
        for sid, TS in ((0, TA), (1, TB)):
            m = pc["st"] == sid
            s_e, d_e, b_e, r_e = pc["s"][m], pc["d"][m], pc["blk"][m], pc["r"][m]
            rc_e = pc["rc"][m]
            idx = np.zeros((c.CPS, 128, c.BPC * TS * 8), np.int16)
            dl = np.full((c.CPS, 128, c.BPC * TS), -1.0, np.float32)
            rc = np.zeros((c.CPS, 128, c.BPC * TS), np.float32)
            ch = b_e // c.BPC
            bi = b_e % c.BPC
            # idx wrap: within (block,stream) sequence position r ->
            # row r%16 (replicated over 16-row bands), col bi*TS*8 + r//16
            rows16 = (r_e % 16).astype(np.int64)
            cols = bi * TS * 8 + r_e // 16
            val = np.where(sid == 0, s_e, s_e - c.SPLIT).astype(np.int16)
            idx[ch, rows16, cols] = val
            idx[:] = np.tile(idx[:, :16, :], (1, 8, 1))
            rowsd = (r_e % 128).astype(np.int64)
            cold = bi * TS + r_e // 128
            dl[ch, rowsd, cold] = (d_e % c.BLK).astype(np.float32)
            rc[ch, rowsd, cold] = rc_e
            sfx = "A" if sid == 0 else "B"
            arrs["idx" + sfx] = idx.reshape(c.CPS * 128, c.BPC * TS * 8)
            arrs["dl" + sfx] = dl.reshape(c.CPS * 128, c.BPC * TS)
            arrs["rc" + sfx] = rc.reshape(c.CPS * 128, c.BPC * TS)
        arrs["zv"] = zpad[None, k * c.NSHARD:(k + 1) * c.NSHARD].copy()
        arrs["npadn"] = np.full((128, 1), float(c.NPAD - c.N) if k == c.NC - 1
                                else 0.0, np.float32)
        prep["cores"].append(arrs)
    return prep


# ---------------------------------------------------------------- builder
def build_program(cfg, TA, TB, dbg=False, phases=9):
    c = cfg
    T, F, H, O = c.T, c.F, c.H, c.O
    BLK, BPC, CHUNK, CPS = c.BLK, c.BPC, c.CHUNK, c.CPS
    NSHARD, TH, TF, GA = c.NSHARD, c.TH, c.TF, c.GA
    HT = H // 128                # 2
    GB = 4 * H // 128            # 8
    NGRP = [(TA, "A"), (TB, "B")]

    nc = bacc.Bacc("TRN2", target_bir_lowering=False, debug=False,
                   enable_asserts=False, num_devices=c.NC)
    dt = lambda n, s, d: nc.dram_tensor(n, s, d, kind="ExternalInput").ap()

    i_xTs = dt("xTs", [T, F, NSHARD], F32)
    i_xall = dt("xall", [c.NPAD, TF], F16)
    i_idxA = dt("idxA", [CPS * 128, BPC * TA * 8], I16)
    i_idxB = dt("idxB", [CPS * 128, BPC * TB * 8], I16)
    i_dlA = dt("dlA", [CPS * 128, BPC * TA], F32)
    i_dlB = dt("dlB", [CPS * 128, BPC * TB], F32)
    i_rcA = dt("rcA", [CPS * 128, BPC * TA], F32)
    i_rcB = dt("rcB", [CPS * 128, BPC * TB], F32)
    i_zv = dt("zv", [1, NSHARD], F32)
    i_iota = dt("iota", [128, BLK], F16)
    i_w0 = dt("w0", [F, H], F32)
    i_b0 = dt("b0v", [H], F32)
    i_wn0xE = dt("wn0xE", [128, H], F16)
    i_wn0xO = dt("wn0xO", [128, H], F16)
    i_b0wn = dt("b0wn", [1, H], F32)
    i_ws0 = dt("ws0", [H, H], F16)
    i_ws1 = dt("ws1", [H, H], F16)
    i_wn1 = dt("wn1", [H, H], F16)
    i_cb = [dt("cb0", [H], F32), dt("cb1", [H], F32)]
    i_g = [dt("g0", [H], F32), dt("g1", [H], F32)]
    i_bt = [dt("bt0", [H], F32), dt("bt1", [H], F32)]
    i_wih = dt("wih", [H, 4 * H], F32)
    i_whh = dt("whh", [H, 4 * H], F32)
    i_bg = dt("bg", [4 * H], F32)
    i_wdec = dt("wdec", [H, O], F32)
    i_bdec = dt("bdec", [O], F32)
    i_npadn = dt("npadn", [128, 1], F32)
    o_out = nc.dram_tensor("out", [O, NSHARD], F32, kind="ExternalOutput").ap()
    if dbg:
        o_dbga = [nc.dram_tensor("dbg_agg0", [128, 4 * CHUNK], F16,
                                 kind="ExternalOutput").ap(),
                  nc.dram_tensor("dbg_agg1", [128, 16 * CHUNK], F16,
                                 kind="ExternalOutput").ap()]
        o_dbgst = nc.dram_tensor("dbg_st", [32, 128], F32,
                                 kind="ExternalOutput").ap()
        o_dbgh = nc.dram_tensor("dbg_h1r", [128, TH], F16,
                                 kind="ExternalOutput").ap()
    grp = [list(range(c.NC))]

    with tile.TileContext(nc) as tc:
        with tc.tile_pool(name="dram", bufs=1, space="DRAM") as dp:
            h0 = dp.tile([T, H, NSHARD], F16)
            h1 = dp.tile([T, H, NSHARD], F16)
            h2 = dp.tile([T, H, NSHARD], F16)
            cmb = dp.tile([T, H, NSHARD], F16)
            h1loc = dp.tile([NSHARD, TH], F16)
            h1all = dp.tile([c.NPAD, TH], F16, addr_space="Shared")
            statsI = dp.tile([32, 128], F32)
            statsO0 = dp.tile([32, 128], F32, addr_space="Shared")
            statsO1 = dp.tile([32, 128], F32, addr_space="Shared")
            statsOL = [statsO0, statsO1]

        from contextlib import ExitStack
        _st = ExitStack()
        cp = _st.enter_context(tc.tile_pool(name="consts", bufs=1))
        ident = cp.tile([128, 128], F32)
        make_identity(nc, ident[:])
        identh = cp.tile([128, 128], F16)
        nc.vector.tensor_copy(out=identh[:], in_=ident[:])
        iota = cp.tile([128, BLK], F16)
        nc.sync.dma_start(out=iota[:], in_=i_iota[:, :])
        w0c = cp.tile([F, H], F32)
        nc.sync.dma_start(out=w0c[:], in_=i_w0[:, :])
        wn0x = [cp.tile([128, H], F16, name=f"wn0x{p}") for p in range(2)]
        nc.sync.dma_start(out=wn0x[0][:], in_=i_wn0xE[:, :])
        nc.sync.dma_start(out=wn0x[1][:], in_=i_wn0xO[:, :])
        b0wn = cp.tile([1, H], F32)
        nc.sync.dma_start(out=b0wn[:], in_=i_b0wn[:, :])
        wsk = {}
        for nm, t_in in (("ws0", i_ws0), ("ws1", i_ws1), ("wn1", i_wn1)):
            for k in range(HT):
                w = cp.tile([128, H], F16, name=f"{nm}k{k}")
                nc.sync.dma_start(out=w[:], in_=t_in[k * 128:(k + 1) * 128, :])
                wsk[nm, k] = w
        b0t, b0h, cbt, gt, btt = {}, {}, {}, {}, {}
        for m in range(HT):
            sl = slice(m * 128, (m + 1) * 128)
            b0t[m] = cp.tile([128, 1], F32, name=f"b0m{m}")
            nc.sync.dma_start(out=b0t[m][:], in_=i_b0[sl, None])
            b0h[m] = cp.tile([128, 1], F16, name=f"b0h{m}")
            nc.vector.tensor_copy(out=b0h[m][:], in_=b0t[m][:])
            for li in range(2):
                cbt[li, m] = cp.tile([128, 1], F32, name=f"cb{li}m{m}")
                nc.sync.dma_start(out=cbt[li, m][:], in_=i_cb[li][sl, None])
                gt[li, m] = cp.tile([128, 1], F32, name=f"g{li}m{m}")
                nc.sync.dma_start(out=gt[li, m][:], in_=i_g[li][sl, None])
                btt[li, m] = cp.tile([128, 1], F32, name=f"bt{li}m{m}")
                nc.sync.dma_start(out=btt[li, m][:], in_=i_bt[li][sl, None])
        npadnT = cp.tile([128, 1], F32)
        nc.sync.dma_start(out=npadnT[:], in_=i_npadn[:, :])

        sacc = _st.enter_context(tc.tile_pool(name="sacc", bufs=1))
        ssum = {(t, m): sacc.tile([128, 1], F32, name=f"ssum{t}_{m}")
                for t in range(T) for m in range(HT)}
        ssq = {(t, m): sacc.tile([128, 1], F32, name=f"ssq{t}_{m}")
               for t in range(T) for m in range(HT)}
        bnsc = {(t, m): sacc.tile([128, 1], F32, name=f"bnsc{t}_{m}")
                for t in range(T) for m in range(HT)}
        bnbi = {(t, m): sacc.tile([128, 1], F32, name=f"bnbi{t}_{m}")
                for t in range(T) for m in range(HT)}
        padh1 = {(t, m): sacc.tile([128, 1], F16, name=f"padh{t}_{m}")
                 for t in range(T) for m in range(HT)}
        epsT = sacc.tile([128, 1], F32, name="epsT")
        nc.vector.memset(epsT[:], BN_EPS)

        def init_stats():
            for t in range(T):
                for m in range(HT):
                    nc.vector.memset(ssum[t, m][:], 0.0)
                    nc.vector.memset(ssq[t, m][:], 0.0)


        # ================= pass1: gather + aggregate + comb + stats
        def pass1(li):
            ELEM = TF if li == 0 else TH            # f16 elems per desc
            table = i_xall if li == 0 else h1all
            hprev = h0 if li == 0 else h1
            idxs = {"A": i_idxA, "B": i_idxB}
            dls = {"A": i_dlA, "B": i_dlB}
            rcs = {"A": i_rcA, "B": i_rcB}
            ga = (max(TA, TB) if li == 0 else GA)   # tiles per gather call
            nacc = 4 if li == 0 else 16

            with tc.tile_pool(name=f"g{li}", bufs=3) as gp, \
                    tc.tile_pool(name=f"w{li}", bufs=2) as wp, \
                    tc.tile_pool(name=f"s{li}", bufs=6) as sp, \
                    tc.tile_pool(name=f"f{li}", bufs=2) as fp, \
                    tc.tile_pool(name=f"ps{li}", bufs=2, space="PSUM") as pp, \
                    tc.tile_pool(name=f"cps{li}", bufs=2, space="PSUM") as cpp:
                for ci in range(CPS):
                    csl = slice(ci * CHUNK, (ci + 1) * CHUNK)
                    r128 = ci * 128
                    st = {}
                    for TS, sfx in NGRP:
                        ix = fp.tile([128, BPC * TS * 8], I16, tag=f"ix{sfx}")
                        nc.sync.dma_start(out=ix[:],
                                          in_=idxs[sfx][r128:r128 + 128, :])
                        dl = fp.tile([128, BPC * TS], F32, tag=f"dl{sfx}")
                        nc.sync.dma_start(out=dl[:],
                                          in_=dls[sfx][r128:r128 + 128, :])
                        rc = fp.tile([128, BPC * TS], F32, tag=f"rc{sfx}")
                        nc.sync.dma_start(out=rc[:],
                                          in_=rcs[sfx][r128:r128 + 128, :])
                        st[sfx] = (ix, dl, rc)
                    hp = {}
                    if li == 0:
                        # fused input projection: h0 chunk computed in SBUF
                        for t in range(T):
                            xc = fp.tile([F, CHUNK], F32, tag=f"xc{t}")
                            nc.sync.dma_start(out=xc[:],
                                              in_=i_xTs[t, :, csl])
                            for m in range(HT):
                                msl = slice(m * 128, (m + 1) * 128)
                                ps0 = cpp.tile([128, CHUNK], F32, tag="p0ps")
                                nc.tensor.matmul(ps0[:], lhsT=w0c[:, msl],
                                                 rhs=xc[:], start=True,
                                                 stop=True)
                                h_ = fp.tile([128, CHUNK], F16,
                                             tag=f"hp{t}{m}")
                                nc.vector.tensor_scalar(h_[:], ps0[:],
                                                        b0t[m][:], 0.0,
                                                        OP.add, OP.add)
                                nc.sync.dma_start(out=h0[t, msl, csl],
                                                  in_=h_[:])
                                hp[t, m] = h_
                    else:
                        for t in range(T):
                            for m in range(HT):
                                h_ = fp.tile([128, CHUNK], F16,
                                             tag=f"hp{t}{m}")
                                nc.sync.dma_start(
                                    out=h_[:],
                                    in_=hprev[t, m * 128:(m + 1) * 128, csl])
                                hp[t, m] = h_
                    agg = wp.tile([128, nacc, CHUNK], F16, tag="agg")
                    for b in range(BPC):
                        acc = pp.tile([128, nacc * BLK], F32, tag="accb",
                                      name="accb")
                        nc.vector.memset(acc[:], 0.0)
                        ntile = TA + TB
                        cur = 0
                        for TS, sfx in NGRP:
                            ix, dl, rc = st[sfx]
                            tb0 = 0 if sfx == "A" else c.SPLIT
                            ncall = (TS + ga - 1) // ga
                            for g in range(ncall):
                                nt = min(ga, TS - g * ga)
                                gtl = gp.tile([128, ga, ELEM], F16, tag="gt")
                                i0 = b * TS * 8 + g * ga * 8
                                nc.gpsimd.dma_gather(
                                    gtl[:, :nt, :], table[tb0:, :],
                                    ix[:, i0:i0 + nt * 8],
                                    nt * 128, nt * 128, ELEM,
                                    single_packet=False)
                                for j in range(nt):
                                    jj = b * TS + g * ga + j
                                    sel = sp.tile([128, BLK], F16, tag="sel")
                                    nc.vector.tensor_scalar(
                                        sel[:], iota[:], dl[:, jj:jj + 1],
                                        rc[:, jj:jj + 1], OP.is_equal, OP.mult)
                                    for i in range(nacc):
                                        nc.tensor.matmul(
                                            acc[:, i * BLK:(i + 1) * BLK],
                                            lhsT=gtl[:, j,
                                                     i * 128:(i + 1) * 128],
                                            rhs=sel[:],
                                            start=False,
                                            stop=(cur == ntile - 1))
                                    cur += 1
                        bs = slice(b * BLK, (b + 1) * BLK)
                        for i in range(nacc):
                            nc.vector.tensor_copy(
                                out=agg[:, i, bs],
                                in_=acc[:, i * BLK:(i + 1) * BLK])
                    if dbg and ci == 0:
                        nc.sync.dma_start(out=o_dbga[li][:, :],
                                          in_=agg[:, :, :])
                    # comb per (t, m)
                    for t in range(T):
                        for m in range(HT):
                            msl = slice(m * 128, (m + 1) * 128)
                            cps = cpp.tile([128, CHUNK], F32, tag="cps")
                            nm = "ws0" if li == 0 else "ws1"
                            nc.tensor.matmul(cps[:], lhsT=wsk[nm, 0][:, msl],
                                             rhs=hp[t, 0][:], start=True,
                                             stop=False)
                            nc.tensor.matmul(cps[:], lhsT=wsk[nm, 1][:, msl],
                                             rhs=hp[t, 1][:], start=False,
                                             stop=False)
                            if li == 0:
                                nc.tensor.matmul(cps[:],
                                                 lhsT=wn0x[t % 2][:, msl],
                                                 rhs=agg[:, t // 2, :],
                                                 start=False, stop=True)
                            else:
                                nc.tensor.matmul(cps[:],
                                                 lhsT=wsk["wn1", 0][:, msl],
                                                 rhs=agg[:, 2 * t, :],
                                                 start=False, stop=False)
                                nc.tensor.matmul(cps[:],
                                                 lhsT=wsk["wn1", 1][:, msl],
                                                 rhs=agg[:, 2 * t + 1, :],
                                                 start=False, stop=True)
                            c16 = wp.tile([128, CHUNK], F16, tag="c16")
                            tsum = sp.tile([128, 1], F32, tag="tsum")
                            nc.vector.tensor_scalar(c16[:], cps[:],
                                                    cbt[li, m][:], 0.0,
                                                    OP.add, OP.add,
                                                    accum_out=tsum[:])
                            nc.vector.tensor_add(ssum[t, m][:], ssum[t, m][:],
                                                 tsum[:])
                            sq = wp.tile([128, CHUNK], F32, tag="sq")
                            tsq = sp.tile([128, 1], F32, tag="tsq")
                            nc.scalar.activation(sq[:], cps[:], AF.Square,
                                                 bias=cbt[li, m][:], scale=1.0,
                                                 accum_out=tsq[:])
                            nc.vector.tensor_add(ssq[t, m][:], ssq[t, m][:],
                                                 tsq[:])
                            nc.sync.dma_start(out=cmb[t, msl, csl],
                                              in_=c16[:])

        # ================= BN finalize with cross-core stats AllReduce
        def bn_ar(li):
            with tc.tile_pool(name=f"bn{li}", bufs=2) as bp, \
                    tc.tile_pool(name=f"bnps{li}", bufs=2, space="PSUM") as bpp:
                nm = "ws0" if li == 0 else "ws1"
                padc = {}
                for t in range(T):
                    for m in range(HT):
                        msl = slice(m * 128, (m + 1) * 128)
                        if li == 0 and t > 0:
                            padc[t, m] = padc[0, m]
                            continue
                        pp_ = bpp.tile([128, 1], F32, tag="pp")
                        ph = ((b0h[0], b0h[1]) if li == 0
                              else (padh1[t, 0], padh1[t, 1]))
                        nc.tensor.matmul(pp_[:], lhsT=wsk[nm, 0][:, msl],
                                         rhs=ph[0][:], start=True, stop=False)
                        nc.tensor.matmul(pp_[:], lhsT=wsk[nm, 1][:, msl],
                                         rhs=ph[1][:], start=False, stop=True)
                        pc = bp.tile([128, 1], F32, name=f"padc{li}_{t}_{m}")
                        nc.vector.tensor_scalar(pc[:], pp_[:], cbt[li, m][:],
                                                0.0, OP.add, OP.add)
                        padc[t, m] = pc
                for t in range(T):
                    for m in range(HT):
                        r = t * HT + m
                        tmp = bp.tile([128, 1], F32, tag="tmp")
                        nc.vector.tensor_tensor(out=tmp[:], in0=padc[t, m][:],
                                                in1=npadnT[:], op=OP.mult)
                        nc.vector.tensor_tensor(out=ssum[t, m][:],
                                                in0=ssum[t, m][:], in1=tmp[:],
                                                op=OP.subtract)
                        sq2 = bp.tile([128, 1], F32, tag="sq2")
                        nc.scalar.activation(sq2[:], padc[t, m][:], AF.Square)
                        nc.vector.tensor_tensor(out=sq2[:], in0=sq2[:],
                                                in1=npadnT[:], op=OP.mult)
                        nc.vector.tensor_tensor(out=ssq[t, m][:],
                                                in0=ssq[t, m][:], in1=sq2[:],
                                                op=OP.subtract)
                        nc.sync.dma_start(out=statsI[r, :, None],
                                          in_=ssum[t, m][:])
                        nc.sync.dma_start(out=statsI[16 + r, :, None],
                                          in_=ssq[t, m][:])
                statsO = statsOL[li]
                nc.gpsimd.collective_compute(
                    "AllReduce", OP.add, replica_groups=grp,
                    ins=[statsI[:, :]], outs=[statsO[:, :]])
                if dbg and li == 0:
                    stile = bp.tile([32, 128], F32, name="stile")
                    nc.sync.dma_start(out=stile[:], in_=statsO[:, :])
                    nc.sync.dma_start(out=o_dbgst[:, :], in_=stile[:])
                for t in range(T):
                    for m in range(HT):
                        r = t * HT + m
                        rsum = bp.tile([128, 1], F32, tag="rsum")
                        nc.sync.dma_start(out=rsum[:], in_=statsO[r, :, None])
                        rsq = bp.tile([128, 1], F32, tag="rsq")
                        nc.sync.dma_start(out=rsq[:],
                                          in_=statsO[16 + r, :, None])
                        mu = bp.tile([128, 1], F32, tag="mu")
                        nc.vector.tensor_scalar(mu[:], rsum[:], 1.0 / c.N,
                                                0.0, OP.mult, OP.add)
                        var = bp.tile([128, 1], F32, tag="var")
                        nc.vector.tensor_scalar(var[:], rsq[:], 1.0 / c.N,
                                                0.0, OP.mult, OP.add)
                        musq = bp.tile([128, 1], F32, tag="musq")
                        nc.vector.tensor_tensor(out=musq[:], in0=mu[:],
                                                in1=mu[:], op=OP.mult)
                        nc.vector.tensor_tensor(out=var[:], in0=var[:],
                                                in1=musq[:], op=OP.subtract)
                        std = bp.tile([128, 1], F32, tag="std")
                        nc.scalar.activation(std[:], var[:], AF.Sqrt,
                                             bias=epsT[:])
                        rstd = bp.tile([128, 1], F32, tag="rstd")
                        nc.vector.reciprocal(rstd[:], std[:])
                        nc.vector.tensor_tensor(out=bnsc[t, m][:],
                                                in0=gt[li, m][:], in1=rstd[:],
                                                op=OP.mult)
                        mt = bp.tile([128, 1], F32, tag="mt")
                        nc.vector.tensor_tensor(out=mt[:], in0=mu[:],
                                                in1=bnsc[t, m][:], op=OP.mult)
                        nc.vector.tensor_tensor(out=bnbi[t, m][:],
                                                in0=btt[li, m][:], in1=mt[:],
                                                op=OP.subtract)
                        pr = bp.tile([128, 1], F32, tag="pr")
                        nc.scalar.activation(pr[:], padc[t, m][:], AF.Relu,
                                             bias=bnbi[t, m][:],
                                             scale=bnsc[t, m][:])
                        ph0 = b0h[m] if li == 0 else padh1[t, m]
                        nc.vector.tensor_add(padh1[t, m][:], ph0[:], pr[:])

        # ================= pass2: h_next = h_prev + relu(BN(cmb))
        def pass2(li):
            hprev = h0 if li == 0 else h1
            hnext = h1 if li == 0 else h2
            with tc.tile_pool(name=f"q{li}", bufs=3) as qp, \
                    tc.tile_pool(name=f"qn{li}", bufs=2) as qn, \
                    tc.tile_pool(name=f"qps{li}", bufs=4, space="PSUM") as qpp:
                for ci in range(CPS):
                    csl = slice(ci * CHUNK, (ci + 1) * CHUNK)
                    hn = {}
                    for t in range(T):
                        for m in range(HT):
                            msl = slice(m * 128, (m + 1) * 128)
                            c16 = qp.tile([128, CHUNK], F16, tag="c16")
                            nc.sync.dma_start(out=c16[:], in_=cmb[t, msl, csl])
                            h_ = qp.tile([128, CHUNK], F16, tag="hpv")
                            nc.sync.dma_start(out=h_[:],
                                              in_=hprev[t, msl, csl])
                            rl = qp.tile([128, CHUNK], F32, tag="rl")
                            nc.scalar.activation(rl[:], c16[:], AF.Relu,
                                                 bias=bnbi[t, m][:],
                                                 scale=bnsc[t, m][:])
                            if li == 0:
                                hx = qn.tile([128, CHUNK], F16,
                                             tag=f"hn{t}{m}")
                            else:
                                hx = qp.tile([128, CHUNK], F16, tag="hx")
                            nc.vector.tensor_add(hx[:], h_[:], rl[:])
                            hn[t, m] = hx
                            nc.sync.dma_start(out=hnext[t, msl, csl],
                                              in_=hx[:])
                    if li == 0:
                        # node-major h1loc rows for the AllGather table
                        for nb in range((CHUNK + 127) // 128):
                            w = min(128, CHUNK - nb * 128)
                            nsl = slice(nb * 128, nb * 128 + w)
                            n16 = qp.tile([128, TH], F16, tag="n16")
                            for t in range(T):
                                for m in range(HT):
                                    tp = qpp.tile([128, 128], F16, tag="tp")
                                    nc.tensor.transpose(
                                        tp[:w, :], hn[t, m][:, nsl],
                                        identh[:])
                                    nc.vector.tensor_copy(
                                        out=n16[:w, t * H + m * 128:
                                                t * H + (m + 1) * 128],
                                        in_=tp[:w, :])
                            r0 = ci * CHUNK + nb * 128
                            nc.sync.dma_start(out=h1loc[r0:r0 + w, :],
                                              in_=n16[:w, :])

        init_stats()
        if phases >= 2:
            pass1(0)
            bn_ar(0)
        if phases >= 3:
            pass2(0)
        if phases >= 4:
            nc.gpsimd.collective_compute(
                "AllGather", OP.bypass, replica_groups=grp,
                ins=[h1loc[:, :]], outs=[h1all[:, :]])
        if dbg:
            with tc.tile_pool(name="dbgh", bufs=1) as dbp:
                ht_ = dbp.tile([128, TH], F16)
                nc.sync.dma_start(out=ht_[:], in_=h1loc[0:128, :])
                nc.sync.dma_start(out=o_dbgh[:, :], in_=ht_[:])
        if phases >= 5:
            init_stats()
            pass1(1)
            bn_ar(1)
        if phases >= 6:
            pass2(1)

        _st.close()

        # ================= LSTM over time + decoder (node-parallel)
        NHALF = NSHARD // 2
        CH = NHALF // CHUNK
        if phases < 7:
            with tc.tile_pool(name="zf", bufs=1) as zf:
                zt = zf.tile([O, CHUNK], F32)
                nc.vector.memset(zt[:], 0.0)
                nc.sync.dma_start(out=o_out[:, :CHUNK], in_=zt[:])
            nc.compile()
            return nc
        with tc.tile_pool(name="lw", bufs=1) as lw, \
                tc.tile_pool(name="lst", bufs=1) as ls, \
                tc.tile_pool(name="lwk", bufs=3) as lk, \
                tc.tile_pool(name="lps", bufs=4, space="PSUM") as lp:
            wih = [lw.tile([128, 4 * H], F16, name=f"wih{k}") for k in range(HT)]
            whh = [lw.tile([128, 4 * H], F16, name=f"whh{k}") for k in range(HT)]
            for k in range(HT):
                nc.gpsimd.dma_start(out=wih[k][:],
                                    in_=i_wih[k * 128:(k + 1) * 128, :])
                nc.gpsimd.dma_start(out=whh[k][:],
                                    in_=i_whh[k * 128:(k + 1) * 128, :])
            bgt = [lw.tile([128, 1], F32, name=f"bg{g}") for g in range(GB)]
            for g in range(GB):
                nc.sync.dma_start(out=bgt[g][:],
                                  in_=i_bg[g * 128:(g + 1) * 128, None])
            bdt = lw.tile([O, 1], F32)
            nc.sync.dma_start(out=bdt[:], in_=i_bdec[:, None])
            wdt = [lw.tile([128, O], F16, name=f"wd{k}") for k in range(HT)]
            for k in range(HT):
                nc.gpsimd.dma_start(out=wdt[k][:],
                                    in_=i_wdec[k * 128:(k + 1) * 128, :])

            cst = [ls.tile([128, NSHARD], F32, name=f"c{m}") for m in range(HT)]
            hst = [ls.tile([128, NSHARD], F16, name=f"h{m}") for m in range(HT)]
            gst = [ls.tile([128, NHALF], F16, name=f"gs{g}") for g in range(GB)]
            eh = [ls.tile([128, NHALF], F16, name=f"e{k}") for k in range(HT)]
            for m in range(HT):
                nc.vector.memset(cst[m][:], 0.0)
                nc.vector.memset(hst[m][:], 0.0)

            for step in range(T):
                for half in range(2):
                    hoff = half * NHALF
                    for k in range(HT):
                        nc.sync.dma_start(
                            out=eh[k][:],
                            in_=h2[step, k * 128:(k + 1) * 128,
                                   hoff:hoff + NHALF])
                    for g in range(GB):
                        gsl = slice(g * 128, (g + 1) * 128)
                        fn = AF.Tanh if g in (4, 5) else AF.Sigmoid
                        for ch in range(CH):
                            s0, s1 = ch * CHUNK, (ch + 1) * CHUNK
                            ps = lp.tile([128, CHUNK], F32, tag="gps")
                            nc.tensor.matmul(ps[:], lhsT=wih[0][:, gsl],
                                             rhs=eh[0][:, s0:s1], start=True,
                                             stop=False)
                            nc.tensor.matmul(ps[:], lhsT=wih[1][:, gsl],
                                             rhs=eh[1][:, s0:s1], start=False,
                                             stop=False)
                            nc.tensor.matmul(
                                ps[:], lhsT=whh[0][:, gsl],
                                rhs=hst[0][:, hoff + s0:hoff + s1],
                                start=False, stop=False)
                            nc.tensor.matmul(
                                ps[:], lhsT=whh[1][:, gsl],
                                rhs=hst[1][:, hoff + s0:hoff + s1],
                                start=False, stop=True)
                            nc.scalar.activation(gst[g][:, s0:s1], ps[:], fn,
                                                 bias=bgt[g][:])
                    for ch in range(CH):
                        s0, s1 = ch * CHUNK, (ch + 1) * CHUNK
                        for m in range(HT):
                            csl_ = cst[m][:, hoff + s0:hoff + s1]
                            t1 = lk.tile([128, CHUNK], F32, tag="t1")
                            nc.vector.tensor_tensor(
                                out=t1[:], in0=gst[2 + m][:, s0:s1],
                                in1=csl_, op=OP.mult)
                            t2 = lk.tile([128, CHUNK], F32, tag="t2")
                            nc.vector.tensor_tensor(
                                out=t2[:], in0=gst[0 + m][:, s0:s1],
                                in1=gst[4 + m][:, s0:s1], op=OP.mult)
                            nc.vector.tensor_tensor(out=csl_, in0=t1[:],
                                                    in1=t2[:], op=OP.add)
                            t3 = lk.tile([128, CHUNK], F32, tag="t3")
                            nc.scalar.activation(t3[:], csl_, AF.Tanh)
                            nc.vector.tensor_tensor(
                                out=hst[m][:, hoff + s0:hoff + s1],
                                in0=gst[6 + m][:, s0:s1], in1=t3[:],
                                op=OP.mult)
            for ci in range(CPS):
                s0, s1 = ci * CHUNK, (ci + 1) * CHUNK
                ps = lp.tile([O, CHUNK], F32, tag="dps")
                nc.tensor.matmul(ps[:], lhsT=wdt[0][:], rhs=hst[0][:, s0:s1],
                                 start=True, stop=False)
                nc.tensor.matmul(ps[:], lhsT=wdt[1][:], rhs=hst[1][:, s0:s1],
                                 start=False, stop=True)
                ob = lk.tile([O, CHUNK], F32, tag="ob")
                nc.vector.tensor_scalar(ob[:], ps[:], bdt[:], 0.0, OP.add,
                                        OP.add)
                nc.sync.dma_start(out=o_out[:, s0:s1], in_=ob[:])

    nc.compile()
    return nc


# ---------------------------------------------------------------- driver
def _make_in_maps(cfg, prep, x, W0, b0, Ws_self, bs_self, Ws_nei, bs_nei,
                  gamma, beta, W_ih, W_hh, b_ih, b_hh, W_dec, b_dec):
    c = cfg
    x = np.asarray(x, np.float32)
    W0 = np.asarray(W0, np.float32)
    b0 = np.asarray(b0, np.float32)
    Ws_nei = np.asarray(Ws_nei, np.float32)
    wn0x = (W0 @ Ws_nei[0]).astype(np.float16)          # [64, 256]
    zr64 = np.zeros((64, c.H), np.float16)
    common = dict(
        xall=None,
        iota=np.broadcast_to(np.arange(c.BLK, dtype=np.float16),
                             (128, c.BLK)).copy(),
        w0=W0, b0v=b0,
        wn0xE=np.concatenate([wn0x, zr64], axis=0),
        wn0xO=np.concatenate([zr64, wn0x], axis=0),
        b0wn=(b0 @ Ws_nei[0])[None, :].astype(np.float32),
        ws0=np.asarray(Ws_self[0], np.float16),
        ws1=np.asarray(Ws_self[1], np.float16),
        wn1=Ws_nei[1].astype(np.float16),
        cb0=(np.asarray(bs_self[0]) + np.asarray(bs_nei[0])).astype(np.float32),
        cb1=(np.asarray(bs_self[1]) + np.asarray(bs_nei[1])).astype(np.float32),
        g0=np.asarray(gamma[0], np.float32),
        g1=np.asarray(gamma[1], np.float32),
        bt0=np.asarray(beta[0], np.float32),
        bt1=np.asarray(beta[1], np.float32),
        wih=np.ascontiguousarray(np.asarray(W_ih, np.float32).T),
        whh=np.ascontiguousarray(np.asarray(W_hh, np.float32).T),
        bg=(np.asarray(b_ih) + np.asarray(b_hh)).astype(np.float32),
        wdec=np.asarray(W_dec, np.float32),
        bdec=np.asarray(b_dec, np.float32),
    )
    # x_all gather table: node-major, all timesteps concat [NPAD, T*F] f16
    xall = np.zeros((c.NPAD, c.TF), np.float16)
    xall[:c.N] = x.transpose(1, 0, 2).reshape(c.N, c.TF).astype(np.float16)
    common["xall"] = xall
    in_maps = []
    for k in range(c.NC):
        lo, hi = k * c.NSHARD, (k + 1) * c.NSHARD
        xs = np.zeros((c.T, c.F, c.NSHARD), np.float32)
        n_real = max(0, min(hi, c.N) - lo)
        if n_real > 0:
            xs[:, :, :n_real] = x[:, lo:lo + n_real, :].transpose(0, 2, 1)
        im = dict(common, xTs=xs, **prep["cores"][k])
        in_maps.append(im)
    return in_maps


def run(cfg, inputs, trace=False):
    prep = host_prep(cfg, inputs["edge_index"])
    nc = build_program(cfg, prep["TA"], prep["TB"])
    in_maps = _make_in_maps(cfg, prep, **{k: v for k, v in inputs.items()
                                          if k != "edge_index"})
    res = bass_utils.run_bass_kernel_spmd(
        nc, in_maps, core_ids=list(range(cfg.NC)), trace=trace)
    outs = [res.results[c]["out"] for c in range(cfg.NC)]
    full = np.concatenate(outs, axis=1)          # [O, NPAD]
    return np.ascontiguousarray(full.T[:cfg.N]), res


def kernel(**inputs):
    out, _ = run(FULL, inputs, trace=bool(os.environ.get("BASS_TRACE")))
    return out.astype(np.float32)
